# revision 6
# baseline (speedup 1.0000x reference)
"""AVWGCN (adaptive-vertex-weight GCN) Trainium2 kernel.

Math (per batch b, node n):
    S = E @ E.T                       [N, N]   (symmetric)
    Mexp = exp(relu(S))               [N, N]   (symmetric)
    r = 1 / rowsum(Mexp)              [N]
    A = diag(r) @ Mexp                (= softmax(relu(S), axis=1))
    z0 = x ;  z1 = diag(r) Mexp z0 ;  z2 = 2 diag(r) Mexp z1 - z0
    y[n, d, o]  = sum_{k,i} z_k[n, i] * Wp[d, k, i, o]
    out[b,n,o]  = sum_d E[n,d] * y[n,d,o] + (E @ bp)[n,o]

Sharding: data-parallel over batch B=64 across 8 cores (B_local=8).
Mexp symmetry lets the same SBUF tile serve as lhsT for Mexp @ V.
"""
import sys

if "/opt/trn_rl_repo" not in sys.path:
    sys.path.insert(0, "/opt/trn_rl_repo")

import numpy as np
import bass_rust
import concourse.bass as bass
import concourse.mybir as mybir
from concourse import tile
from concourse.vector_clock import ScopedClock
from concourse.bass_utils import run_bass_kernel_spmd

F32 = mybir.dt.float32
F32R = mybir.dt.float32r
ALU = mybir.AluOpType
AF = mybir.ActivationFunctionType

B, N, CI, CO, KCH, D = 64, 2048, 64, 64, 3, 10
NCORES = 8
BL = B // NCORES          # 8 local batches
BC = BL * CI              # 512
NT = N // 128             # 16 node chunks
DO = D * CO               # 640


# ---------------------------------------------------------------- env patches
def _patched_drain_and_barrier(self, tick_clock, wait_clock):
    """Tail drain: walrus here allows only one sync-wait per instruction, so
    put each wait on its own sync nop before the drain."""
    nop_inst = self.nc.sync.nop(nofuse=True, hint="tile_tail_wait")
    wait_clock.add_sem_waits(nop_inst.ins,
                             ScopedClock({None: tick_clock.global_clock}))
    si = nop_inst.ins.sync_info
    waits = list(si.on_wait) if si is not None else []
    if len(waits) > 1:
        nop_inst.ins.sync_info = bass_rust.SyncInfo(
            on_wait=waits[:1], on_update=list(si.on_update))
        for w in waits[1:]:
            extra = self.nc.sync.nop(nofuse=True, hint="tile_tail_wait_x")
            extra.ins.sync_info = bass_rust.SyncInfo(on_wait=[w], on_update=[])
    self.nc.sync.drain()
    self.nc.all_engine_barrier()
    assert self.sems is not None
    popped = self.nc._tile_sem_poison_stack.pop()
    assert popped is self._sem_poison
    self.nc.clear_and_free_semaphores(list(self.sems.allocated().values()))
    self.nc.all_engine_barrier()


tile.TileContext._drain_and_barrier = _patched_drain_and_barrier


def split_multi_waits(nc):
    """Hoist extra sync-waits onto same-engine NoOps (1-wait walrus cap)."""
    for f in nc.m.functions:
        for bb in f.blocks:
            new = []
            for inst in bb.instructions:
                si = inst.sync_info
                if si is not None and len(si.on_wait) > 1:
                    waits = list(si.on_wait)
                    for w in waits[:-1]:
                        nop = mybir.InstNoOp(
                            name=nc.get_next_instruction_name(), ins=[], outs=[])
                        nop.engine = inst.engine
                        nop.sync_info = bass_rust.SyncInfo(on_wait=[w], on_update=[])
                        new.append(nop)
                    inst.sync_info = bass_rust.SyncInfo(
                        on_wait=[waits[-1]], on_update=list(si.on_update))
                new.append(inst)
            bb.instructions = new


# ---------------------------------------------------------------- kernel body
def build_nc():
    nc = bass.Bass()

    x_l = nc.dram_tensor("x_l", [BL, N, CI], F32, kind="ExternalInput")
    emb = nc.dram_tensor("emb", [N, D], F32, kind="ExternalInput")
    wp = nc.dram_tensor("wp", [D, KCH, CI, CO], F32, kind="ExternalInput")
    bp = nc.dram_tensor("bp", [D, CO], F32, kind="ExternalInput")
    ident = nc.dram_tensor("ident", [128, 128], F32, kind="ExternalInput")
    out_l = nc.dram_tensor("out_l", [BL, N, CI], F32, kind="ExternalOutput")

    with tile.TileContext(nc) as tc:
        with (
            tc.tile_pool(name="dram", bufs=1, space="DRAM") as dpool,
            tc.tile_pool(name="const", bufs=1) as cpool,
            tc.tile_pool(name="xr", bufs=1) as xrpool,
            tc.tile_pool(name="z1r", bufs=1) as z1pool,
            tc.tile_pool(name="stage", bufs=4) as stpool,
            tc.tile_pool(name="mcol", bufs=3) as mcpool,
        ):
            mexp_d = dpool.tile([N, N], F32R, name="mexp_d")
            zt_d = [dpool.tile([BC, N], F32R, name=f"zt_d{k}") for k in range(KCH)]

            e_sb = cpool.tile([128, NT * D], F32)       # [p, nt*10+d]
            et_sb = cpool.tile([D, N], F32)             # E^T
            bias_sb = cpool.tile([128, NT * CO], F32)   # [p, nt*64+o]
            bp_sb = cpool.tile([D, CO], F32)
            id_r = cpool.tile([128, 128], F32R)
            id_f = cpool.tile([128, 128], F32)
            rowsum = cpool.tile([128, NT], F32)
            rinv = cpool.tile([128, NT], F32)
            r2 = cpool.tile([128, NT], F32)
            wr_dup = [cpool.tile([128, DO], F32R, name=f"wr{k}", tag=f"wr{k}") for k in range(KCH)]

            x_r = [xrpool.tile([128, BC], F32R, name=f"x{t}", tag=f"x{t}") for t in range(NT)]
            z1_r = [z1pool.tile([128, BC], F32R, name=f"z1{t}", tag=f"z1{t}") for t in range(NT)]

            with tc.tile_pool(name="pstr", bufs=2, space="PSUM") as pstr:
                # ---- constants / embeddings -------------------------------
                nc.sync.dma_start(id_f[:], ident[:])
                nc.vector.tensor_copy(id_r[:], id_f[:])
                for nt in range(NT):
                    nc.sync.dma_start(e_sb[:, nt * D:(nt + 1) * D],
                                      emb[nt * 128:(nt + 1) * 128, :])
                for nt in range(NT):
                    pt = pstr.tile([128, 128], F32, tag="ptr")
                    nc.tensor.transpose(pt[0:D, 0:128],
                                        e_sb[:, nt * D:(nt + 1) * D], id_f[:])
                    nc.scalar.copy(et_sb[:, nt * 128:(nt + 1) * 128],
                                   pt[0:D, 0:128])

                nc.sync.dma_start(bp_sb[:], bp[:])
                for nt in range(NT):
                    pb = pstr.tile([128, 128], F32, tag="ptr")
                    nc.tensor.matmul(pb[:, 0:CO],
                                     et_sb[:, nt * 128:(nt + 1) * 128],
                                     bp_sb[:], start=True, stop=True)
                    nc.scalar.copy(bias_sb[:, nt * CO:(nt + 1) * CO], pb[:, 0:CO])

                # Wp as rhs [i, (d,o)] duplicated on both partition halves
                for k in range(KCH):
                    wf = stpool.tile([128, DO], F32, tag="wload")
                    src = wp[:, k, :, :].transpose([1, 0, 2])   # [i, d, o]
                    nc.sync.dma_start(
                        wf[0:CI, :].rearrange("p (d o) -> p d o", d=D), src)
                    nc.sync.dma_start(
                        wf[CI:128, :].rearrange("p (d o) -> p d o", d=D), src)
                    nc.vector.tensor_copy(wr_dup[k][:], wf[:])

                # ---- phase 1: Mexp = exp(relu(E E^T)), rowsums ------------
                with (
                    tc.tile_pool(name="ps1", bufs=2, space="PSUM") as ps1,
                    tc.tile_pool(name="msb", bufs=2) as mpool,
                ):
                    for nt in range(NT):
                        lhs = et_sb[:, nt * 128:(nt + 1) * 128]
                        m_f = mpool.tile([128, N], F32, tag="m_f")
                        for half in range(2):
                            ps = ps1.tile([128, 1024], F32)
                            for j in range(2):
                                c0 = half * 1024 + j * 512
                                nc.tensor.matmul(ps[:, j * 512:(j + 1) * 512],
                                                 lhs, et_sb[:, c0:c0 + 512],
                                                 start=True, stop=True)
                            nc.vector.tensor_scalar_max(
                                m_f[:, half * 1024:(half + 1) * 1024], ps[:], 0.0)
                        m_r = mpool.tile([128, N], F32R, tag="m_r")
                        nc.scalar.activation(m_r[:], m_f[:], AF.Exp,
                                             accum_out=rowsum[:, nt:nt + 1])
                        nc.sync.dma_start(mexp_d[nt * 128:(nt + 1) * 128, :],
                                          m_r[:])

                nc.vector.reciprocal(rinv[:], rowsum[:])
                nc.vector.tensor_scalar_mul(r2[:], rinv[:], 2.0)

                # ---- phase 2: load X as [m, (b,c)] f32r -------------------
                for mt in range(NT):
                    xf = stpool.tile([128, BC], F32, tag="xload")
                    nc.sync.dma_start(
                        xf[:].rearrange("p (b c) -> p b c", b=BL),
                        x_l[:, mt * 128:(mt + 1) * 128, :].transpose([1, 0, 2]))
                    nc.vector.tensor_copy(x_r[mt][:], xf[:])

                def transposes_to_dram(src_tile, k, nt, eng_flip):
                    """4 PE transposes [128,128]: z_k[nt] -> zt_d[k] col block."""
                    for w in range(4):
                        ptr = pstr.tile([128, 128], F32R, tag="ptr")
                        nc.tensor.transpose(ptr[:],
                                            src_tile[:, w * 128:(w + 1) * 128],
                                            id_r[:])
                        zt_st = stpool.tile([128, 128], F32R, tag="zt_st")
                        if (w + eng_flip) % 2 == 0:
                            nc.scalar.copy(zt_st[:], ptr[:])
                        else:
                            nc.vector.tensor_copy(zt_st[:], ptr[:])
                        nc.sync.dma_start(
                            zt_d[k][w * 128:(w + 1) * 128,
                                    nt * 128:(nt + 1) * 128],
                            zt_st[:])

                # ---- phase 3: z1 = diag(r) Mexp X ; z0T, z1T --------------
                with tc.tile_pool(name="psmm", bufs=2, space="PSUM") as psmm:
                    for nt in range(NT):
                        mcol = mcpool.tile([128, N], F32R, tag="mcol")
                        nc.sync.dma_start(
                            mcol[:].rearrange("p (mt c) -> p mt c", mt=NT),
                            mexp_d[:, nt * 128:(nt + 1) * 128]
                            .rearrange("(mt p) c -> p mt c", p=128))
                        ps = psmm.tile([128, BC], F32)
                        for mt in range(NT):
                            nc.tensor.matmul(ps[:],
                                             mcol[:, mt * 128:(mt + 1) * 128],
                                             x_r[mt][:], start=(mt == 0),
                                             stop=(mt == NT - 1))
                        nc.scalar.activation(z1_r[nt][:], ps[:], AF.Copy,
                                             scale=rinv[:, nt:nt + 1])
                        transposes_to_dram(x_r[nt], 0, nt, 0)
                        transposes_to_dram(z1_r[nt], 1, nt, 1)

                    # ---- phase 4: z2 = 2 diag(r) Mexp z1 - x ; z2T --------
                    for nt in range(NT):
                        mcol = mcpool.tile([128, N], F32R, tag="mcol")
                        nc.sync.dma_start(
                            mcol[:].rearrange("p (mt c) -> p mt c", mt=NT),
                            mexp_d[:, nt * 128:(nt + 1) * 128]
                            .rearrange("(mt p) c -> p mt c", p=128))
                        ps = psmm.tile([128, BC], F32)
                        for mt in range(NT):
                            nc.tensor.matmul(ps[:],
                                             mcol[:, mt * 128:(mt + 1) * 128],
                                             z1_r[mt][:], start=(mt == 0),
                                             stop=(mt == NT - 1))
                        z2t = stpool.tile([128, BC], F32R, tag="z2")
                        nc.vector.scalar_tensor_tensor(
                            z2t[:], ps[:], r2[:, nt:nt + 1], x_r[nt][:],
                            op0=ALU.mult, op1=ALU.subtract)
                        transposes_to_dram(z2t, 2, nt, 0)

            # ---- phase 5: y = zT . Wp ; out = sum_d E_d * y_d + bias ------
            with (
                tc.tile_pool(name="psy", bufs=2, space="PSUM") as psy,
                tc.tile_pool(name="zslab", bufs=2) as zspool,
                tc.tile_pool(name="accp", bufs=2) as accpool,
            ):
                for nt in range(NT):
                    slabs = []
                    for k in range(KCH):
                        sl = zspool.tile([128, BC], F32R, tag=f"sl{k}")
                        nc.sync.dma_start(
                            sl[:].rearrange("p (w c) -> p w c", w=4),
                            zt_d[k][:, nt * 128:(nt + 1) * 128]
                            .rearrange("(w p) c -> p w c", p=128))
                        slabs.append(sl)
                    acc = accpool.tile([128, BC], F32, tag="acc")
                    accv = acc[:].rearrange("p (q o) -> p q o", q=BL)
                    # pre-fill acc with bias (same for every b)
                    for b in range(BL):
                        nc.scalar.copy(acc[:, b * CO:(b + 1) * CO],
                                       bias_sb[:, nt * CO:(nt + 1) * CO])
                    for bp2 in range(BL // 2):          # b pairs
                        ps = psy.tile([128, 2048], F32)
                        for bh in range(2):
                            b = bp2 * 2 + bh
                            po = bh * 1024
                            hp = (b % 2) * 64
                            wc = (b // 2) * 128
                            for k in range(KCH):
                                lhsT = slabs[k][hp:hp + 64, wc:wc + 128]
                                nc.tensor.matmul(
                                    ps[:, po:po + 512],
                                    lhsT, wr_dup[k][hp:hp + 64, 0:512],
                                    start=(k == 0), stop=(k == KCH - 1))
                                nc.tensor.matmul(
                                    ps[:, po + 512:po + DO],
                                    lhsT, wr_dup[k][hp:hp + 64, 512:DO],
                                    start=(k == 0), stop=(k == KCH - 1))
                        # d-contraction over both b of the pair at once
                        pair = ps[:].rearrange("p (b f) -> p b f", b=2)
                        o01 = accv[:, bp2 * 2:bp2 * 2 + 2, :]
                        for d in range(D):
                            nc.vector.scalar_tensor_tensor(
                                o01, pair[:, :, d * CO:(d + 1) * CO],
                                e_sb[:, nt * D + d:nt * D + d + 1], o01,
                                op0=ALU.mult, op1=ALU.add)
                    nc.sync.dma_start(
                        out_l[:, nt * 128:(nt + 1) * 128, :].transpose([1, 0, 2]),
                        accv)

    split_multi_waits(nc)
    return nc


_NC_CACHE = None


def get_nc():
    global _NC_CACHE
    if _NC_CACHE is None:
        _NC_CACHE = build_nc()
    return _NC_CACHE


def make_in_maps(inputs):
    x = np.ascontiguousarray(np.asarray(inputs["x"], dtype=np.float32))
    emb = np.ascontiguousarray(np.asarray(inputs["node_embeddings"],
                                          dtype=np.float32))
    wpa = np.ascontiguousarray(np.asarray(inputs["weights_pool"],
                                          dtype=np.float32))
    bpa = np.ascontiguousarray(np.asarray(inputs["bias_pool"],
                                          dtype=np.float32))
    ident = np.eye(128, dtype=np.float32)
    return [dict(x_l=x[c * BL:(c + 1) * BL], emb=emb, wp=wpa, bp=bpa,
                 ident=ident) for c in range(NCORES)]


def kernel(**inputs) -> np.ndarray:
    nc = get_nc()
    res = run_bass_kernel_spmd(nc, make_in_maps(inputs), list(range(NCORES)))
    out = np.concatenate([res.results[c]["out_l"] for c in range(NCORES)],
                         axis=0)
    return out.astype(np.float32)
